# revision 1
# baseline (speedup 1.0000x reference)
"""Multi-head attention (RoPE + causal-mask softmax) on 8 TRN2 NeuronCores.

Sharding: batch x q-chunk (2 batches x 4 chunks of 512 query rows). Each core
computes all 16 heads for its 512 query rows. K/V are recomputed per core
(whole batch), so there are no collectives; outputs are disjoint slices.

To make the program SPMD-uniform, each core's sequence axis is rolled so its
query window sits at s=0 (attention is permutation-invariant over the key
axis when K, V, mask and the RoPE tables are rolled together).
"""

from contextlib import ExitStack

import numpy as np

import concourse.bass as bass
import concourse.tile as tile
from concourse import bacc, mybir
from concourse.alu_op_type import AluOpType
from concourse.bass_utils import run_bass_kernel_spmd

AF = mybir.ActivationFunctionType
F32 = mybir.dt.float32
F32R = mybir.dt.float32r
F16 = mybir.dt.float16
BF16 = mybir.dt.bfloat16

B, S, HID, NH, HD = 2, 2048, 1024, 16, 64
SCALE = 1.0 / np.sqrt(HD)
N_CORES = 8
Q = 512          # query rows per core
HC = HID // 128  # hidden chunks (8)
PAIRS = NH // 2  # head pairs (8)
SC16 = S // 128  # key chunks of 128 (16)
SC4 = S // 512   # key chunks of 512 (4)


def build_program(dbg: bool = False):
    nc = bacc.Bacc("TRN2", target_bir_lowering=False, debug=False,
                   num_devices=N_CORES)

    hsT = nc.dram_tensor("hsT", [HID, S], BF16, kind="ExternalInput").ap()
    cosk = nc.dram_tensor("cosk", [128, S], BF16, kind="ExternalInput").ap()
    sink = nc.dram_tensor("sink", [128, S], BF16, kind="ExternalInput").ap()
    emask = nc.dram_tensor("emask", [S, Q], F16, kind="ExternalInput").ap()
    wq = nc.dram_tensor("wq", [HID, HID], BF16, kind="ExternalInput").ap()
    wk = nc.dram_tensor("wk", [HID, HID], BF16, kind="ExternalInput").ap()
    wv = nc.dram_tensor("wv", [HID, HID], BF16, kind="ExternalInput").ap()
    wo = nc.dram_tensor("wo", [HID, HID], F16, kind="ExternalInput").ap()
    sel = nc.dram_tensor("sel", [128, HID], F32R, kind="ExternalInput").ap()
    out = nc.dram_tensor("out", [Q, HID], F32, kind="ExternalOutput").ap()
    if dbg:
        d_kt = nc.dram_tensor("d_kt", [128, S], F32, kind="ExternalOutput").ap()
        d_qt = nc.dram_tensor("d_qt", [128, Q], F32, kind="ExternalOutput").ap()
        d_acc = nc.dram_tensor("d_acc", [128, Q], F32, kind="ExternalOutput").ap()
        d_v = nc.dram_tensor("d_v", [128, NH * 65], F32, kind="ExternalOutput").ap()

    with tile.TileContext(nc) as tc, ExitStack() as top:
        res = top.enter_context(tc.tile_pool(name="res", bufs=1))

        # ---- resident tiles -------------------------------------------------
        hsT_sb = []
        for hc in range(HC):
            t = res.tile([128, S], BF16, tag=f"hsT{hc}")
            nc.sync.dma_start(t[:], hsT[hc * 128:(hc + 1) * 128, :])
            hsT_sb.append(t)
        cos_sb = res.tile([128, S], BF16, tag="cos")
        nc.scalar.dma_start(cos_sb[:], cosk[:])
        sin_sb = res.tile([128, S], BF16, tag="sin")
        nc.scalar.dma_start(sin_sb[:], sink[:])
        em_sb = []
        for sc in range(8):
            t = res.tile([128, 2 * Q], F16, tag=f"em{sc}")
            em3 = t[:].rearrange("p (c q) -> p c q", c=2)
            nc.scalar.dma_start(em3[:, 0, :], emask[sc * 256:sc * 256 + 128, :])
            nc.scalar.dma_start(em3[:, 1, :], emask[sc * 256 + 128:sc * 256 + 256, :])
            em_sb.append(t)
        # V_aug: per key-chunk, 16 heads x (64 cols + ones col)
        v_sb = [res.tile([128, NH * 65], F16, tag=f"v{sc}", name=f"v{sc}")
                for sc in range(SC16)]
        ones16 = res.tile([128, NH], F16, tag="ones16")
        nc.gpsimd.memset(ones16[:], 1.0)
        # per-pair attention output accumulator [hd(128), q] (unnormalized)
        acc_sb = [res.tile([128, Q], F16, tag=f"acc{p}", name=f"acc{p}")
                  for p in range(PAIRS)]
        den_all = res.tile([128, Q], F32, tag="den_all")
        nc.gpsimd.memset(den_all[:], 1.0)
        wk_sb, wq_sb = [], []
        for hc in range(HC):
            t = res.tile([128, HID], BF16, tag=f"wkr{hc}", name=f"wkr{hc}")
            nc.gpsimd.dma_start(t[:], wk[hc * 128:(hc + 1) * 128, :])
            wk_sb.append(t)
            t = res.tile([128, HID], BF16, tag=f"wqr{hc}", name=f"wqr{hc}")
            nc.gpsimd.dma_start(t[:], wq[hc * 128:(hc + 1) * 128, :])
            wq_sb.append(t)

        # ---- V projection ---------------------------------------------------
        with tc.tile_pool(name="wvp", bufs=1) as wvp, \
             tc.tile_pool(name="psv", bufs=4, space="PSUM") as psv:
            for g in range(2):  # groups of 8 heads = 512 cols
                wv_g = []
                for hc in range(HC):
                    t = wvp.tile([128, 512], BF16, tag=f"wv{hc}")
                    nc.scalar.dma_start(
                        t[:], wv[hc * 128:(hc + 1) * 128, g * 512:(g + 1) * 512])
                    wv_g.append(t)
                for sc in range(SC16):
                    ps = psv.tile([128, 512], F32, tag="psv")
                    for hc in range(HC):
                        nc.tensor.matmul(
                            ps[:], hsT_sb[hc][:, sc * 128:(sc + 1) * 128],
                            wv_g[hc][:], start=(hc == 0), stop=(hc == HC - 1))
                    v3 = v_sb[sc][:].rearrange("p (h c) -> p h c", h=NH)
                    ps3 = ps[:].rearrange("p (h c) -> p h c", h=8)
                    nc.scalar.copy(v3[:, 8 * g:8 * g + 8, 0:64], ps3[:])
                # ones columns for these heads
                for sc in range(SC16):
                    v3 = v_sb[sc][:].rearrange("p (h c) -> p h c", h=NH)
                    nc.gpsimd.tensor_copy(v3[:, 8 * g:8 * g + 8, 64],
                                          ones16[:, 8 * g:8 * g + 8])

        # ---- head-pair loop -------------------------------------------------
        with tc.tile_pool(name="wqk", bufs=2) as wqk, \
             tc.tile_pool(name="kt", bufs=2) as ktp, \
             tc.tile_pool(name="qt", bufs=2) as qtp, \
             tc.tile_pool(name="rope", bufs=2) as rope, \
             tc.tile_pool(name="expp", bufs=3) as expp, \
             tc.tile_pool(name="nrm", bufs=2) as nrm, \
             tc.tile_pool(name="pss", bufs=2, space="PSUM") as pss, \
             tc.tile_pool(name="psk", bufs=2, space="PSUM") as psk, \
             tc.tile_pool(name="psa", bufs=1, space="PSUM") as psa:

            def rope_apply(dst, ps, d0, s0, n):
                """dst[:, d0:d0+n] = rope(ps) using table cols [s0, s0+n).

                sin_sb is pre-shifted+signed on host so the rotate-half
                quarter products use aligned input bases (shifted output)."""
                with nc.allow_low_precision(reason="bf16 rope"):
                    kraw = rope.tile([128, 512], BF16, tag="kraw")
                    nc.scalar.copy(kraw[:, :n], ps[:, :n])
                    t1 = rope.tile([128, 512], BF16, tag="t1")
                    nc.vector.tensor_tensor(
                        t1[:, :n], kraw[:, :n], cos_sb[:, s0:s0 + n],
                        AluOpType.mult)
                    t2 = rope.tile([128, 512], BF16, tag="t2")
                    for hb in (0, 64):
                        nc.gpsimd.tensor_tensor(
                            t2[hb:hb + 32, :n], kraw[hb + 32:hb + 64, :n],
                            sin_sb[hb + 32:hb + 64, s0:s0 + n], AluOpType.mult)
                        nc.gpsimd.tensor_tensor(
                            t2[hb + 32:hb + 64, :n], kraw[hb:hb + 32, :n],
                            sin_sb[hb:hb + 32, s0:s0 + n], AluOpType.mult)
                    nc.vector.tensor_tensor(
                        dst[:, d0:d0 + n], t1[:, :n], t2[:, :n], AluOpType.add)

            for p in range(PAIRS):
                c0 = p * 128

                # K projection + RoPE -> kT pair-packed [128, S]
                kt_h2 = [ktp.tile([128, S // 2], BF16, tag=f"kt{i}",
                                  name=f"kt{i}") for i in range(2)]
                for sc in range(SC4):
                    ps = psk.tile([128, 512], F32, tag="psk")
                    for hc in range(HC):
                        nc.tensor.matmul(
                            ps[:], wk_sb[hc][:, c0:c0 + 128],
                            hsT_sb[hc][:, sc * 512:(sc + 1) * 512],
                            start=(hc == 0), stop=(hc == HC - 1))
                    rope_apply(kt_h2[sc // 2], ps, (sc % 2) * 512, sc * 512, 512)

                # Q projection + RoPE -> qT pair-packed [128, Q]
                qt_pr = qtp.tile([128, Q], BF16, tag="qt")
                ps = psk.tile([128, 512], F32, tag="psk")
                for hc in range(HC):
                    nc.tensor.matmul(ps[:], wq_sb[hc][:, c0:c0 + 128],
                                     hsT_sb[hc][:, 0:Q],
                                     start=(hc == 0), stop=(hc == HC - 1))
                rope_apply(qt_pr, ps, 0, 0, Q)

                if dbg and p == 0:
                    nc.gpsimd.dma_start(d_kt[:, 0:S // 2], kt_h2[0][:])
                    nc.gpsimd.dma_start(d_kt[:, S // 2:], kt_h2[1][:])
                    nc.gpsimd.dma_start(d_qt[:], qt_pr[:])

                # attention per head
                ps_aa = [psa.tile([65, Q], F32, tag=f"psaA{half}",
                                  name=f"psaA{half}") for half in range(2)]
                ps_ab = [psa.tile([65, Q], F32, tag=f"psaB{half}",
                                  name=f"psaB{half}") for half in range(2)]
                t_ems = {}
                for sc2 in range(8):
                    t_exps = [expp.tile([128, 2 * Q], F16, tag=f"texp{half}",
                                        name=f"texp{half}") for half in range(2)]
                    for j in range(2):
                        sc = 2 * sc2 + j
                        for half in range(2):
                            hb = half * 64
                            ps_s = pss.tile([128, Q], F32, tag="pss")
                            kth = kt_h2[sc // 8]
                            scc = sc % 8
                            nc.tensor.matmul(
                                ps_s[:],
                                kth[hb:hb + 64, scc * 128:(scc + 1) * 128],
                                qt_pr[hb:hb + 64, :], start=True, stop=True)
                            nc.scalar.activation(
                                t_exps[half][:, j * Q:(j + 1) * Q], ps_s[:],
                                AF.Exp)
                    for half in range(2):
                        h = 2 * p + half
                        t_em = expp.tile([128, 2 * Q], F16, tag=f"tem{half}",
                                         name=f"tem{half}")
                        nc.vector.tensor_tensor(t_em[:], t_exps[half][:],
                                                em_sb[sc2][:], AluOpType.mult)
                        t_ems[half] = t_em
                    for j in range(2):
                        sc = 2 * sc2 + j
                        for half in range(2):
                            h = 2 * p + half
                            nc.tensor.matmul(
                                ps_aa[half][:],
                                v_sb[sc][0:64, h * 65:h * 65 + 65],
                                t_ems[half][0:64, j * Q:(j + 1) * Q],
                                start=(sc == 0), stop=(sc == SC16 - 1))
                            nc.tensor.matmul(
                                ps_ab[half][:],
                                v_sb[sc][64:128, h * 65:h * 65 + 65],
                                t_ems[half][64:128, j * Q:(j + 1) * Q],
                                start=(sc == 0), stop=(sc == SC16 - 1))
                for half in range(2):
                    hb = half * 64
                    h = 2 * p + half
                    t_ab = nrm.tile([65, Q], F32, tag="t_ab")
                    nc.scalar.copy(t_ab[:], ps_ab[half][:])
                    dtmp = nrm.tile([1, Q], F32, tag="dtmp")
                    nc.vector.tensor_tensor(dtmp[:], ps_aa[half][64:65, :],
                                            t_ab[64:65, :], AluOpType.add)
                    nc.sync.dma_start(den_all[h:h + 1, :], dtmp[:])
                    with nc.allow_low_precision(reason="fp16 attn accum"):
                        nc.vector.tensor_tensor(acc_sb[p][hb:hb + 64, :],
                                                ps_aa[half][0:64, :],
                                                t_ab[0:64, :], AluOpType.add)

        if dbg:
            with tc.tile_pool(name="dbgp", bufs=1) as dbgp:
                t_d3 = dbgp.tile([128, Q], F32, tag="td3")
                nc.vector.tensor_copy(t_d3[:], acc_sb[0][:])
                nc.sync.dma_start(d_acc[:], t_d3[:])
                t_d4 = dbgp.tile([128, NH * 65], F32, tag="td4")
                nc.vector.tensor_copy(t_d4[:], v_sb[0][:])
                nc.sync.dma_start(d_v[:], t_d4[:])

        # ---- normalize (deferred) + output projection -----------------------
        with tc.tile_pool(name="wop", bufs=1) as wop, \
             tc.tile_pool(name="outp", bufs=3) as outp, \
             tc.tile_pool(name="nrm2", bufs=1) as nrm2, \
             tc.tile_pool(name="psb", bufs=2, space="PSUM") as psb, \
             tc.tile_pool(name="pso", bufs=2, space="PSUM") as pso:
            sel_sb = nrm2.tile([128, HID], F32R, tag="sel")
            nc.sync.dma_start(sel_sb[:], sel[:])
            recip_all = nrm2.tile([128, Q], F32R, tag="recip")
            with nc.allow_low_precision(reason="f32r reciprocal broadcast"):
                nc.vector.reciprocal(recip_all[:], den_all[:])
            acc2 = []
            for p in range(PAIRS):
                ps_bc = psb.tile([128, Q], F32, tag="psb")
                nc.tensor.matmul(ps_bc[:], sel_sb[:, p * 128:(p + 1) * 128],
                                 recip_all[:], start=True, stop=True)
                a2 = nrm2.tile([128, Q], F16, tag=f"acc2_{p}", name=f"acc2_{p}")
                with nc.allow_low_precision(reason="fp16 attention weights"):
                    nc.vector.tensor_tensor(a2[:], acc_sb[p][:], ps_bc[:],
                                            AluOpType.mult)
                acc2.append(a2)
            wo_p = []
            for p in range(PAIRS):
                t = wop.tile([128, HID], F16, tag=f"wo{p}")
                nc.gpsimd.dma_start(t[:], wo[p * 128:(p + 1) * 128, :])
                wo_p.append(t)
            for qc in range(Q // 128):
                for nn in range(2):
                    ps = pso.tile([128, 512], F32, tag="pso")
                    for p in range(PAIRS):
                        nc.tensor.matmul(
                            ps[:], acc2[p][:, qc * 128:(qc + 1) * 128],
                            wo_p[p][:, nn * 512:(nn + 1) * 512],
                            start=(p == 0), stop=(p == PAIRS - 1))
                    t_out = outp.tile([128, 512], F32, tag="tout")
                    nc.vector.tensor_copy(t_out[:], ps[:])
                    nc.sync.dma_start(
                        out[qc * 128:(qc + 1) * 128, nn * 512:(nn + 1) * 512],
                        t_out[:])

    nc.compile()
    return nc


_NC_CACHE = None


def _get_program():
    global _NC_CACHE
    if _NC_CACHE is None:
        _NC_CACHE = build_program()
    return _NC_CACHE


def make_in_maps(hidden_states, attention_mask, position_ids, cos, sin,
                 Wq, Wk, Wv, Wo):
    import ml_dtypes
    bf16 = ml_dtypes.bfloat16
    hidden_states = np.asarray(hidden_states, np.float32)
    attention_mask = np.asarray(attention_mask, np.float32)
    position_ids = np.asarray(position_ids)
    cos = np.asarray(cos, np.float32)
    sin = np.asarray(sin, np.float32)
    wq_s = (np.asarray(Wq, np.float32) * SCALE).astype(bf16)
    wk_ = np.ascontiguousarray(np.asarray(Wk, np.float32)).astype(bf16)
    wv_ = np.ascontiguousarray(np.asarray(Wv, np.float32)).astype(bf16)
    wo_ = np.ascontiguousarray(np.asarray(Wo, np.float32)).astype(np.float16)

    sel = np.zeros((128, HID), np.float32)
    for p in range(PAIRS):
        for m in range(128):
            sel[2 * p + (m >= 64), 128 * p + m] = 1.0

    in_maps = []
    for b in range(B):
        hsT_b = hidden_states[b].T  # [HID, S]
        cos_b = cos[position_ids[b]]  # [S, HD]
        sin_b = sin[position_ids[b]]
        cosT = np.tile(cos_b.T, (2, 1)).astype(bf16)  # [128, S]
        # pre-shifted + signed sin: row q holds sign(swap(q)) * sin[swap(q)]
        # where swap flips 32-halves within each 64-block (rotate_half)
        sin64 = sin_b.T  # [64, S]
        sh = np.empty_like(sin64)
        sh[0:32] = sin64[32:64]          # swap(q)=q+32 (>=32) -> sign +1
        sh[32:64] = -sin64[0:32]         # swap(q)=q-32 (<32)  -> sign -1
        sinT = np.tile(sh, (2, 1)).astype(bf16)  # [128, S]
        maskT_b = attention_mask[b, 0].T  # [S(keys), S(queries)]
        for qc in range(4):
            q0 = qc * Q
            roll = -q0
            in_maps.append({
                "hsT": np.ascontiguousarray(np.roll(hsT_b, roll, axis=1)).astype(bf16),
                "cosk": np.ascontiguousarray(np.roll(cosT, roll, axis=1)),
                "sink": np.ascontiguousarray(np.roll(sinT, roll, axis=1)),
                "emask": np.exp(
                    np.roll(maskT_b[:, q0:q0 + Q], roll, axis=0)
                ).astype(np.float16),
                "wq": wq_s, "wk": wk_, "wv": wv_, "wo": wo_, "sel": sel,
            })
    return in_maps


def run(inputs: dict, trace: bool = False):
    nc = _get_program()
    in_maps = make_in_maps(**inputs)
    res = run_bass_kernel_spmd(nc, in_maps, list(range(N_CORES)), trace=trace)
    out = np.empty((B, S, HID), np.float32)
    for c in range(N_CORES):
        b, qc = c // 4, c % 4
        out[b, qc * Q:(qc + 1) * Q, :] = res.results[c]["out"]
    return out, res


def kernel(**inputs) -> np.ndarray:
    out, _ = run(inputs, trace=False)
    return out



# revision 5
# speedup vs baseline: 1.0908x; 1.0908x over previous
"""Multi-head attention (RoPE + causal-mask softmax) on 8 TRN2 NeuronCores.

Sharding: batch x head-group (2 batches x 4 groups of 4 heads). Each core
computes q/k/v projections for its 4 heads over the full sequence, attention
for all 2048 queries, and a partial o_proj; a per-query-block ReduceScatter
over the 4 cores of each batch sums the o_proj partials and scatters query
rows, so each core ends with 4 disjoint 128-row strips of the final output.

Head-sharding keeps the program SPMD-uniform while letting the causal
structure skip score blocks above the block diagonal (every core sees the
same query/key trapezoid). kernel() inspects the mask at runtime: if it is
(effectively) causal it builds the trapezoid program, otherwise a full-mask
fallback program.
"""

from contextlib import ExitStack

import numpy as np

import concourse.bass as bass
import concourse.tile as tile
from concourse import bacc, mybir
from concourse.alu_op_type import AluOpType
from concourse.bass_utils import run_bass_kernel_spmd

AF = mybir.ActivationFunctionType
F32 = mybir.dt.float32
F16 = mybir.dt.float16
BF16 = mybir.dt.bfloat16

B, S, HID, NH, HD = 2, 2048, 1024, 16, 64
SCALE = 1.0 / np.sqrt(HD)
N_CORES = 8
HPC = 4            # heads per core
CPB = 4            # cores per batch
HC = HID // 128    # hidden chunks (8)
QC = S // 512      # query chunks of 512 (4)
KC = S // 128      # key chunks of 128 (16)
GROUPS = [[0, 1, 2, 3], [4, 5, 6, 7]]


def build_program(causal: bool):
    nc = bacc.Bacc("TRN2", target_bir_lowering=False, debug=False,
                   num_devices=N_CORES)

    hsT = nc.dram_tensor("hsT", [HID, S], BF16, kind="ExternalInput").ap()
    cosk = nc.dram_tensor("cosk", [128, S], BF16, kind="ExternalInput").ap()
    sink = nc.dram_tensor("sink", [128, S], BF16, kind="ExternalInput").ap()
    # causal: exp(mask) diag blocks, [keys 512 per qc stacked, q 512 x2 dup]
    # general: exp(mask) full, [keys S, q S]
    em_cols = 1024 if causal else S
    emask = nc.dram_tensor("emask", [S, em_cols], F16, kind="ExternalInput").ap()
    wq = nc.dram_tensor("wq", [HID, HPC * HD], BF16, kind="ExternalInput").ap()
    wk = nc.dram_tensor("wk", [HID, HPC * HD], BF16, kind="ExternalInput").ap()
    wv = nc.dram_tensor("wv", [HID, HPC * HD], BF16, kind="ExternalInput").ap()
    wo = nc.dram_tensor("wo", [HPC * HD, HID], F16, kind="ExternalInput").ap()
    out = nc.dram_tensor("out", [512, HID], F32, kind="ExternalOutput").ap()

    with tile.TileContext(nc) as tc, ExitStack() as top:
        res = top.enter_context(tc.tile_pool(name="res", bufs=1))
        dram = top.enter_context(tc.tile_pool(name="dram", bufs=1, space="DRAM"))

        po = [dram.tile([512, HID], F32, tag=f"po{qc}", name=f"po{qc}")
              for qc in range(QC)]
        rs_o = [dram.tile([128, HID], F32, tag=f"rso{qc}", name=f"rso{qc}")
                for qc in range(QC)]

        # ---- resident tiles -------------------------------------------------
        hsT_sb = []
        for hc in range(HC):
            t = res.tile([128, S], BF16, tag=f"hsT{hc}")
            for sc in range(4):
                nc.sync.dma_start(t[:, sc * 512:(sc + 1) * 512],
                                  hsT[hc * 128:(hc + 1) * 128,
                                      sc * 512:(sc + 1) * 512])
            hsT_sb.append(t)
        cos_sb = res.tile([128, S], BF16, tag="cos")
        nc.scalar.dma_start(cos_sb[:], cosk[:])
        sin_sb = res.tile([128, S], BF16, tag="sin")
        nc.scalar.dma_start(sin_sb[:], sink[:])
        wq_sb, wk_sb, wv_sb = [], [], []
        for hc in range(HC):
            for lst, w, tg in ((wq_sb, wq, "wq"), (wk_sb, wk, "wk"),
                               (wv_sb, wv, "wv")):
                t = res.tile([128, HPC * HD], BF16, tag=f"{tg}{hc}")
                nc.gpsimd.dma_start(t[:], w[hc * 128:(hc + 1) * 128, :])
                lst.append(t)
        wo_p = []
        for p in range(2):
            t = res.tile([128, HID], F16, tag=f"wo{p}")
            nc.gpsimd.dma_start(t[:], wo[p * 128:(p + 1) * 128, :])
            wo_p.append(t)
        # exp(mask) tiles per query chunk
        em_sb = []
        if causal:
            for qc in range(QC):
                t = res.tile([128, 4 * 1024], F16, tag=f"em{qc}")
                em3 = t[:].rearrange("p (c q) -> p c q", c=4)
                for kc in range(4):
                    r0 = qc * 512 + kc * 128
                    nc.scalar.dma_start(em3[:, kc, :], emask[r0:r0 + 128, :])
                em_sb.append(t)
        else:
            for qc in range(QC):
                t = res.tile([128, KC * 512], F16, tag=f"em{qc}")
                em3 = t[:].rearrange("p (c q) -> p c q", c=KC)
                for kc in range(KC):
                    nc.scalar.dma_start(
                        em3[:, kc, :],
                        emask[kc * 128:(kc + 1) * 128,
                              qc * 512:(qc + 1) * 512])
                em_sb.append(t)
        # K/Q pair-packed [head dims: pair head A 0:64, head B 64:128]
        kt = [res.tile([128, S], BF16, tag=f"kt{p}", name=f"kt{p}")
              for p in range(2)]
        qt = [res.tile([128, S], BF16, tag=f"qt{p}", name=f"qt{p}")
              for p in range(2)]
        # V augmented: per key-chunk, 4 heads x (64 cols + ones col)
        v_sb = [res.tile([128, HPC * 65], F16, tag=f"v{kc}", name=f"v{kc}")
                for kc in range(KC)]
        ones4 = res.tile([128, HPC], F16, tag="ones4")
        nc.gpsimd.memset(ones4[:], 1.0)
        # normalized attention output, pair-packed
        acc2 = [res.tile([128, S], F16, tag=f"acc2_{p}", name=f"acc2_{p}")
                for p in range(2)]

        def rope_apply(dst, ps, d0, s0, n, rope):
            """dst[:, d0:d0+n] = rope(ps) using table cols [s0, s0+n).

            sin_sb is pre-shifted+signed on host so the rotate-half
            quarter products use aligned input bases."""
            with nc.allow_low_precision(reason="bf16 rope"):
                kraw = rope.tile([128, 512], BF16, tag="kraw")
                nc.scalar.copy(kraw[:, :n], ps[:, :n])
                t1 = rope.tile([128, 512], BF16, tag="t1")
                nc.vector.tensor_tensor(
                    t1[:, :n], kraw[:, :n], cos_sb[:, s0:s0 + n],
                    AluOpType.mult)
                t2 = rope.tile([128, 512], BF16, tag="t2")
                for hb in (0, 64):
                    nc.gpsimd.tensor_tensor(
                        t2[hb:hb + 32, :n], kraw[hb + 32:hb + 64, :n],
                        sin_sb[hb + 32:hb + 64, s0:s0 + n], AluOpType.mult)
                    nc.gpsimd.tensor_tensor(
                        t2[hb + 32:hb + 64, :n], kraw[hb:hb + 32, :n],
                        sin_sb[hb:hb + 32, s0:s0 + n], AluOpType.mult)
                nc.vector.tensor_tensor(
                    dst[:, d0:d0 + n], t1[:, :n], t2[:, :n], AluOpType.add)

        # ---- K/Q projection + RoPE -----------------------------------------
        with tc.tile_pool(name="rope", bufs=2) as rope, \
             tc.tile_pool(name="psk", bufs=2, space="PSUM") as psk:
            for p in range(2):
                for sc in range(4):
                    ps = psk.tile([128, 512], F32, tag="psk")
                    for hc in range(HC):
                        nc.tensor.matmul(
                            ps[:], wk_sb[hc][:, p * 128:(p + 1) * 128],
                            hsT_sb[hc][:, sc * 512:(sc + 1) * 512],
                            start=(hc == 0), stop=(hc == HC - 1))
                    rope_apply(kt[p], ps, sc * 512, sc * 512, 512, rope)
            for p in range(2):
                for sc in range(4):
                    ps = psk.tile([128, 512], F32, tag="psk")
                    for hc in range(HC):
                        nc.tensor.matmul(
                            ps[:], wq_sb[hc][:, p * 128:(p + 1) * 128],
                            hsT_sb[hc][:, sc * 512:(sc + 1) * 512],
                            start=(hc == 0), stop=(hc == HC - 1))
                    rope_apply(qt[p], ps, sc * 512, sc * 512, 512, rope)

        # ---- V projection ---------------------------------------------------
        with tc.tile_pool(name="psv", bufs=2, space="PSUM") as psv:
            for kc in range(KC):
                ps = psv.tile([128, HPC * HD], F32, tag="psv")
                for hc in range(HC):
                    nc.tensor.matmul(
                        ps[:], hsT_sb[hc][:, kc * 128:(kc + 1) * 128],
                        wv_sb[hc][:], start=(hc == 0), stop=(hc == HC - 1))
                v3 = v_sb[kc][:].rearrange("p (h c) -> p h c", h=HPC)
                ps3 = ps[:].rearrange("p (h c) -> p h c", h=HPC)
                with nc.allow_low_precision(reason="fp16 v"):
                    nc.scalar.copy(v3[:, :, 0:64], ps3[:])
                nc.gpsimd.tensor_copy(v3[:, :, 64], ones4[:])

        # ---- attention + o_proj, per query chunk ---------------------------
        with tc.tile_pool(name="expp", bufs=3) as expp, \
             tc.tile_pool(name="nrm", bufs=2) as nrm, \
             tc.tile_pool(name="outp", bufs=2) as outp, \
             tc.tile_pool(name="pss", bufs=2, space="PSUM") as pss, \
             tc.tile_pool(name="psa", bufs=1, space="PSUM") as psa, \
             tc.tile_pool(name="pso", bufs=2, space="PSUM") as pso:
            for qc in range(QC):
                n_kc = 4 * (qc + 1) if causal else KC
                for p in range(2):
                    ps_a = [psa.tile([65, 512], F32, tag=f"psa{h}",
                                     name=f"psa{h}") for h in range(2)]
                    for kc in range(n_kc):
                        pse = pss.tile([128, 1024], F32, tag="pse")
                        for half in range(2):
                            hb = half * 64
                            nc.tensor.matmul(
                                pse[:, half * 512:(half + 1) * 512],
                                kt[p][hb:hb + 64, kc * 128:(kc + 1) * 128],
                                qt[p][hb:hb + 64, qc * 512:(qc + 1) * 512],
                                start=True, stop=True)
                        tex = expp.tile([128, 1024], F16, tag="tex")
                        nc.scalar.activation(tex[:], pse[:], AF.Exp)
                        if causal and kc >= 4 * qc:
                            tem = expp.tile([128, 1024], F16, tag="tem")
                            nc.vector.tensor_tensor(
                                tem[:], tex[:],
                                em_sb[qc][:, (kc - 4 * qc) * 1024:
                                          (kc - 4 * qc + 1) * 1024],
                                AluOpType.mult)
                        elif not causal:
                            tem = expp.tile([128, 1024], F16, tag="tem")
                            for half in range(2):
                                nc.vector.tensor_tensor(
                                    tem[:, half * 512:(half + 1) * 512],
                                    tex[:, half * 512:(half + 1) * 512],
                                    em_sb[qc][:, kc * 512:(kc + 1) * 512],
                                    AluOpType.mult)
                        else:
                            tem = tex
                        for half in range(2):
                            h = 2 * p + half
                            nc.tensor.matmul(
                                ps_a[half][:],
                                v_sb[kc][:, h * 65:h * 65 + 65],
                                tem[:, half * 512:(half + 1) * 512],
                                start=(kc == 0), stop=(kc == n_kc - 1))
                    # normalize: acc2 = ps_a[0:64] / ps_a[64]
                    for half in range(2):
                        hb = half * 64
                        rec = nrm.tile([1, 512], F32, tag="rec")
                        with nc.allow_low_precision(reason="recip bcast"):
                            nc.vector.reciprocal(rec[:], ps_a[half][64:65, :])
                        recb = nrm.tile([64, 512], F32, tag="recb")
                        nc.gpsimd.partition_broadcast(recb[:], rec[:])
                        with nc.allow_low_precision(reason="fp16 attn out"):
                            nc.vector.tensor_tensor(
                                acc2[p][hb:hb + 64, qc * 512:(qc + 1) * 512],
                                ps_a[half][0:64, :], recb[:], AluOpType.mult)
                # o_proj partials for this query chunk
                for qch in range(4):
                    q0 = qc * 512 + qch * 128
                    for nn in range(2):
                        ps = pso.tile([128, 512], F32, tag="pso")
                        for p in range(2):
                            nc.tensor.matmul(
                                ps[:], acc2[p][:, q0:q0 + 128],
                                wo_p[p][:, nn * 512:(nn + 1) * 512],
                                start=(p == 0), stop=(p == 1))
                        t_out = outp.tile([128, 512], F32, tag="tout")
                        nc.vector.tensor_copy(t_out[:], ps[:])
                        nc.sync.dma_start(
                            po[qc][qch * 128:(qch + 1) * 128,
                                   nn * 512:(nn + 1) * 512], t_out[:])
                # sum partials over the batch group; rank r gets 128 rows
                nc.gpsimd.collective_compute(
                    "ReduceScatter", AluOpType.add, replica_groups=GROUPS,
                    ins=[po[qc].opt()], outs=[rs_o[qc].opt()])
                nc.sync.dma_start(out[qc * 128:(qc + 1) * 128, :],
                                  rs_o[qc][:])

    nc.compile()
    return nc


_NC_CACHE = {}


def _get_program(causal: bool = True):
    if causal not in _NC_CACHE:
        _NC_CACHE[causal] = build_program(causal)
    return _NC_CACHE[causal]


def _detect_causal(attention_mask):
    """True if everything at or above the 512-block diagonal's upper edge is
    masked off hard enough that exp(mask) == 0 for our purposes."""
    m = np.asarray(attention_mask)  # [B, 1, S(q), S(k)]
    for qc in range(QC):
        k0 = (qc + 1) * 512
        if k0 >= S:
            continue
        blk = m[:, 0, qc * 512:(qc + 1) * 512, k0:]
        if not np.all(blk < -30.0):
            return False
    return True


def make_in_maps(hidden_states, attention_mask, position_ids, cos, sin,
                 Wq, Wk, Wv, Wo, causal):
    import ml_dtypes
    bf16 = ml_dtypes.bfloat16
    hidden_states = np.asarray(hidden_states, np.float32)
    attention_mask = np.asarray(attention_mask, np.float32)
    position_ids = np.asarray(position_ids)
    cos = np.asarray(cos, np.float32)
    sin = np.asarray(sin, np.float32)
    wq_f = np.asarray(Wq, np.float32) * SCALE
    wk_f = np.asarray(Wk, np.float32)
    wv_f = np.asarray(Wv, np.float32)
    wo_f = np.asarray(Wo, np.float32)

    in_maps = []
    for b in range(B):
        hsT_b = np.ascontiguousarray(hidden_states[b].T).astype(bf16)
        cos_b = cos[position_ids[b]]  # [S, HD]
        sin_b = sin[position_ids[b]]
        cosT = np.tile(cos_b.T, (2, 1)).astype(bf16)  # [128, S]
        # pre-shifted + signed sin: row q holds sign(swap(q)) * sin[swap(q)]
        sin64 = sin_b.T  # [64, S]
        sh = np.empty_like(sin64)
        sh[0:32] = sin64[32:64]
        sh[32:64] = -sin64[0:32]
        sinT = np.tile(sh, (2, 1)).astype(bf16)  # [128, S]
        mask_b = attention_mask[b, 0]  # [S(q), S(k)]
        if causal:
            em = np.empty((S, 1024), np.float16)
            for qc in range(QC):
                blk = mask_b[qc * 512:(qc + 1) * 512,
                             qc * 512:(qc + 1) * 512].T  # [k, q]
                e = np.exp(blk).astype(np.float16)
                em[qc * 512:(qc + 1) * 512, 0:512] = e
                em[qc * 512:(qc + 1) * 512, 512:1024] = e
        else:
            em = np.exp(mask_b.T).astype(np.float16)  # [k, q]
        for g in range(CPB):
            c0 = g * HPC * HD
            in_maps.append({
                "hsT": hsT_b, "cosk": cosT, "sink": sinT, "emask": em,
                "wq": np.ascontiguousarray(wq_f[:, c0:c0 + HPC * HD]).astype(bf16),
                "wk": np.ascontiguousarray(wk_f[:, c0:c0 + HPC * HD]).astype(bf16),
                "wv": np.ascontiguousarray(wv_f[:, c0:c0 + HPC * HD]).astype(bf16),
                "wo": np.ascontiguousarray(wo_f[c0:c0 + HPC * HD, :]).astype(np.float16),
            })
    return in_maps


def run(inputs: dict, trace: bool = False):
    causal = _detect_causal(inputs["attention_mask"])
    nc = _get_program(causal)
    in_maps = make_in_maps(**inputs, causal=causal)
    res = run_bass_kernel_spmd(nc, in_maps, list(range(N_CORES)), trace=trace)
    out = np.empty((B, S, HID), np.float32)
    for c in range(N_CORES):
        b, r = c // CPB, c % CPB
        for qc in range(QC):
            q0 = qc * 512 + r * 128
            out[b, q0:q0 + 128, :] = res.results[c]["out"][qc * 128:(qc + 1) * 128]
    return out, res


def kernel(**inputs) -> np.ndarray:
    out, _ = run(inputs, trace=False)
    return out


# revision 19
# speedup vs baseline: 1.5445x; 1.4159x over previous
"""Multi-head attention (RoPE + causal-mask softmax) on 8 TRN2 NeuronCores.

Sharding: batch x head-group (2 batches x 4 groups of 4 heads). Each core
computes q/k/v projections for its 4 heads over the full sequence and
attention for all 2048 queries. Per 512-query chunk, an AllToAll over the
4 cores of the batch exchanges normalized attention outputs so each core
ends up with all 16 heads for a disjoint 128-query strip, then runs the
full o_proj locally (no partial-sum reduction collective needed).

Head-sharding keeps the program SPMD-uniform while letting the causal
structure skip score blocks above the block diagonal (every core sees the
same query/key trapezoid). kernel() inspects the mask at runtime: if it is
(effectively) causal it builds the trapezoid program, otherwise a full-mask
fallback program.
"""

from contextlib import ExitStack

import numpy as np

import concourse.bass as bass
import concourse.tile as tile
from concourse import bacc, mybir
from concourse.alu_op_type import AluOpType
from concourse.bass_utils import run_bass_kernel_spmd

AF = mybir.ActivationFunctionType
F32 = mybir.dt.float32
F16 = mybir.dt.float16
BF16 = mybir.dt.bfloat16

B, S, HID, NH, HD = 2, 2048, 1024, 16, 64
SCALE = 1.0 / np.sqrt(HD)
N_CORES = 8
HPC = 4            # heads per core
CPB = 4            # cores per batch
HC = HID // 128    # hidden chunks (8)
QC = S // 512      # query chunks of 512 (4)
KC = S // 128      # key chunks of 128 (16)
GROUPS = [[0, 1, 2, 3], [4, 5, 6, 7]]


def build_program(causal: bool):
    nc = bacc.Bacc("TRN2", target_bir_lowering=False, debug=False,
                   num_devices=N_CORES)

    hsT = nc.dram_tensor("hsT", [HID, S], BF16, kind="ExternalInput").ap()
    cosk = nc.dram_tensor("cosk", [128, S], BF16, kind="ExternalInput").ap()
    sink = nc.dram_tensor("sink", [128, S], BF16, kind="ExternalInput").ap()
    # causal: exp(mask) diag blocks, [keys 512 per qc stacked, q 512 x2 dup]
    # general: exp(mask) full, [keys S, q S]
    em_cols = 1024 if causal else S
    emask = nc.dram_tensor("emask", [S, em_cols], F16, kind="ExternalInput").ap()
    wq = nc.dram_tensor("wq", [HID, HPC * HD], BF16, kind="ExternalInput").ap()
    wk = nc.dram_tensor("wk", [HID, HPC * HD], BF16, kind="ExternalInput").ap()
    wv = nc.dram_tensor("wv", [HID, HPC * HD], BF16, kind="ExternalInput").ap()
    wo = nc.dram_tensor("wo", [HID, HID], F16, kind="ExternalInput").ap()
    out = nc.dram_tensor("out", [512, HID], F32, kind="ExternalOutput").ap()

    with tile.TileContext(nc) as tc, ExitStack() as top:
        res = top.enter_context(tc.tile_pool(name="res", bufs=1))
        dram = top.enter_context(tc.tile_pool(name="dram", bufs=1, space="DRAM"))

        # AllGather exchange buffers, one per query chunk: each core
        # contributes its normalized [2 pairs x 128, 512 q] block; after the
        # gather, rows [g*256 + p*128] hold peer g's pair-p heads and every
        # core reads its own 128-query column window (rank-dynamic offset).
        ag_in = [dram.tile([256, 512], F16, tag=f"ai{qc}", name=f"ai{qc}")
                 for qc in range(QC)]
        ag_out = [dram.tile([CPB * 256, 512], F16, tag=f"ao{qc}",
                            name=f"ao{qc}") for qc in range(QC)]

        # ---- resident tiles -------------------------------------------------
        hsT_sb = []
        for hc in range(HC):
            t = res.tile([128, S], BF16, tag=f"hsT{hc}")
            for sc in range(4):
                nc.sync.dma_start(t[:, sc * 512:(sc + 1) * 512],
                                  hsT[hc * 128:(hc + 1) * 128,
                                      sc * 512:(sc + 1) * 512])
            hsT_sb.append(t)
        cos_sb = res.tile([128, S], BF16, tag="cos")
        nc.scalar.dma_start(cos_sb[:], cosk[:])
        sin_sb = res.tile([128, S], BF16, tag="sin")
        nc.scalar.dma_start(sin_sb[:], sink[:])
        wq_sb, wk_sb, wv_sb = [], [], []
        for hc in range(HC):
            for lst, w, tg in ((wq_sb, wq, "wq"), (wk_sb, wk, "wk"),
                               (wv_sb, wv, "wv")):
                t = res.tile([128, HPC * HD], BF16, tag=f"{tg}{hc}")
                nc.gpsimd.dma_start(t[:], w[hc * 128:(hc + 1) * 128, :])
                lst.append(t)
        # full Wo (o_proj runs over all 16 heads after the exchange)
        wo_sb = []
        for hb in range(HC):
            t = res.tile([128, HID], F16, tag=f"wo{hb}")
            nc.gpsimd.dma_start(t[:], wo[hb * 128:(hb + 1) * 128, :])
            wo_sb.append(t)
        # exp(mask) tiles per query chunk
        em_sb = []
        if causal:
            for qc in range(QC):
                t = res.tile([128, 4 * 1024], F16, tag=f"em{qc}")
                em3 = t[:].rearrange("p (c q) -> p c q", c=4)
                for kc in range(4):
                    r0 = qc * 512 + kc * 128
                    nc.scalar.dma_start(em3[:, kc, :], emask[r0:r0 + 128, :])
                em_sb.append(t)
        else:
            for qc in range(QC):
                t = res.tile([128, KC * 512], F16, tag=f"em{qc}")
                em3 = t[:].rearrange("p (c q) -> p c q", c=KC)
                for kc in range(KC):
                    nc.scalar.dma_start(
                        em3[:, kc, :],
                        emask[kc * 128:(kc + 1) * 128,
                              qc * 512:(qc + 1) * 512])
                em_sb.append(t)
        # K/Q pair-packed [head dims: pair head A 0:64, head B 64:128]
        kt = [res.tile([128, S], BF16, tag=f"kt{p}", name=f"kt{p}")
              for p in range(2)]
        qt = [res.tile([128, S], BF16, tag=f"qt{p}", name=f"qt{p}")
              for p in range(2)]
        # V augmented: per key-chunk, 4 heads x (64 cols + ones col)
        v_sb = [res.tile([128, HPC * 65], F16, tag=f"v{kc}", name=f"v{kc}")
                for kc in range(KC)]
        ones4 = res.tile([128, HPC], F16, tag="ones4")
        nc.gpsimd.memset(ones4[:], 1.0)
        # normalized attention output, pair-packed
        acc2 = [res.tile([128, S], F16, tag=f"acc2_{p}", name=f"acc2_{p}")
                for p in range(2)]

        def rope_apply(dst, ps, d0, s0, n, rope):
            """dst[:, d0:d0+n] = rope(ps) using table cols [s0, s0+n).

            sin_sb is pre-shifted+signed on host so the rotate-half
            quarter products use aligned input bases."""
            with nc.allow_low_precision(reason="bf16 rope"):
                kraw = rope.tile([128, 512], BF16, tag="kraw")
                nc.scalar.copy(kraw[:, :n], ps[:, :n])
                # rotate-half partition swap (32-blocks) via SBUF->SBUF DMA
                # (DMA is the engine that can move data across partitions)
                ksw = rope.tile([128, 512], BF16, tag="ksw")
                for hb in (0, 64):
                    nc.sync.dma_start(ksw[hb:hb + 32, :n],
                                      kraw[hb + 32:hb + 64, :n])
                    nc.sync.dma_start(ksw[hb + 32:hb + 64, :n],
                                      kraw[hb:hb + 32, :n])
                t1 = rope.tile([128, 512], BF16, tag="t1")
                nc.vector.tensor_tensor(
                    t1[:, :n], kraw[:, :n], cos_sb[:, s0:s0 + n],
                    AluOpType.mult)
                t2 = rope.tile([128, 512], BF16, tag="t2")
                nc.vector.tensor_tensor(
                    t2[:, :n], ksw[:, :n], sin_sb[:, s0:s0 + n],
                    AluOpType.mult)
                nc.vector.tensor_tensor(
                    dst[:, d0:d0 + n], t1[:, :n], t2[:, :n], AluOpType.add)

        # ---- K/Q projection + RoPE -----------------------------------------
        with tc.tile_pool(name="rope", bufs=2) as rope, \
             tc.tile_pool(name="psk", bufs=2, space="PSUM") as psk:
            for p in range(2):
                for sc in range(4):
                    ps = psk.tile([128, 512], F32, tag="psk")
                    for hc in range(HC):
                        nc.tensor.matmul(
                            ps[:], wk_sb[hc][:, p * 128:(p + 1) * 128],
                            hsT_sb[hc][:, sc * 512:(sc + 1) * 512],
                            start=(hc == 0), stop=(hc == HC - 1))
                    rope_apply(kt[p], ps, sc * 512, sc * 512, 512, rope)
            for p in range(2):
                for sc in range(4):
                    ps = psk.tile([128, 512], F32, tag="psk")
                    for hc in range(HC):
                        nc.tensor.matmul(
                            ps[:], wq_sb[hc][:, p * 128:(p + 1) * 128],
                            hsT_sb[hc][:, sc * 512:(sc + 1) * 512],
                            start=(hc == 0), stop=(hc == HC - 1))
                    rope_apply(qt[p], ps, sc * 512, sc * 512, 512, rope)

        # ---- V projection ---------------------------------------------------
        with tc.tile_pool(name="psv", bufs=2, space="PSUM") as psv:
            for kc in range(KC):
                ps = psv.tile([128, HPC * HD], F32, tag="psv")
                for hc in range(HC):
                    nc.tensor.matmul(
                        ps[:], hsT_sb[hc][:, kc * 128:(kc + 1) * 128],
                        wv_sb[hc][:], start=(hc == 0), stop=(hc == HC - 1))
                v3 = v_sb[kc][:].rearrange("p (h c) -> p h c", h=HPC)
                ps3 = ps[:].rearrange("p (h c) -> p h c", h=HPC)
                with nc.allow_low_precision(reason="fp16 v"):
                    nc.scalar.copy(v3[:, :, 0:64], ps3[:])
                nc.gpsimd.tensor_copy(v3[:, :, 64], ones4[:])

        # ---- attention + exchange + o_proj, per query chunk -----------------
        with tc.tile_pool(name="expp", bufs=3) as expp, \
             tc.tile_pool(name="nrm", bufs=2) as nrm, \
             tc.tile_pool(name="gath", bufs=2) as gathp, \
             tc.tile_pool(name="outp", bufs=2) as outp, \
             tc.tile_pool(name="pss", bufs=2, space="PSUM") as pss, \
             tc.tile_pool(name="psa", bufs=1, space="PSUM") as psa, \
             tc.tile_pool(name="pso", bufs=2, space="PSUM") as pso:
            # our rank's query-column window within each gathered chunk
            col0 = (nc.scalar.partition_id() % CPB) * 128
            for qc in range(QC):
                n_kc = 4 * (qc + 1) if causal else KC
                for p in range(2):
                    ps_a = [psa.tile([65, 512], F32, tag=f"psa{h}",
                                     name=f"psa{h}") for h in range(2)]
                    for kc in range(n_kc):
                        pse = pss.tile([128, 1024], F32, tag="pse")
                        for half in range(2):
                            hb = half * 64
                            nc.tensor.matmul(
                                pse[:, half * 512:(half + 1) * 512],
                                kt[p][hb:hb + 64, kc * 128:(kc + 1) * 128],
                                qt[p][hb:hb + 64, qc * 512:(qc + 1) * 512],
                                start=True, stop=True)
                        tex = expp.tile([128, 1024], F16, tag="tex")
                        nc.scalar.activation(tex[:], pse[:], AF.Exp)
                        if causal and kc >= 4 * qc:
                            tem = expp.tile([128, 1024], F16, tag="tem")
                            nc.vector.tensor_tensor(
                                tem[:], tex[:],
                                em_sb[qc][:, (kc - 4 * qc) * 1024:
                                          (kc - 4 * qc + 1) * 1024],
                                AluOpType.mult)
                        elif not causal:
                            tem = expp.tile([128, 1024], F16, tag="tem")
                            for half in range(2):
                                nc.vector.tensor_tensor(
                                    tem[:, half * 512:(half + 1) * 512],
                                    tex[:, half * 512:(half + 1) * 512],
                                    em_sb[qc][:, kc * 512:(kc + 1) * 512],
                                    AluOpType.mult)
                        else:
                            tem = tex
                        for half in range(2):
                            h = 2 * p + half
                            nc.tensor.matmul(
                                ps_a[half][:],
                                v_sb[kc][:, h * 65:h * 65 + 65],
                                tem[:, half * 512:(half + 1) * 512],
                                start=(kc == 0), stop=(kc == n_kc - 1))
                    for half in range(2):
                        hb = half * 64
                        # den lives at PSUM partition 64; hop it to partition
                        # 0 (32-aligned cross-partition copy is legal), recip
                        # there, then broadcast (which always reads part. 0)
                        rec0 = nrm.tile([1, 512], F32, tag="rec0")
                        nc.vector.tensor_copy(rec0[:], ps_a[half][64:65, :])
                        rect = nrm.tile([1, 512], F32, tag="rect")
                        nc.vector.reciprocal_approx_fast(rect[:], rec0[:])
                        recb = nrm.tile([64, 512], F32, tag="recb")
                        nc.gpsimd.partition_broadcast(recb[:], rect[:])
                        with nc.allow_low_precision(reason="fp16 attn out"):
                            nc.vector.tensor_tensor(
                                acc2[p][hb:hb + 64, qc * 512:(qc + 1) * 512],
                                ps_a[half][0:64, :], recb[:], AluOpType.mult)
                # ship this chunk's normalized outputs and gather all heads
                for p in range(2):
                    nc.sync.dma_start(
                        ag_in[qc][p * 128:(p + 1) * 128, :],
                        acc2[p][:, qc * 512:(qc + 1) * 512])
                nc.gpsimd.collective_compute(
                    "AllGather", AluOpType.bypass, replica_groups=GROUPS,
                    ins=[ag_in[qc].opt()], outs=[ag_out[qc].opt()])
                # all 16 heads for our rank's 128 queries, then o_proj
                gath = gathp.tile([128, 8 * 128], F16, tag="gath")
                for hb in range(HC):
                    nc.scalar.dma_start(
                        gath[:, hb * 128:(hb + 1) * 128],
                        ag_out[qc][hb * 128:(hb + 1) * 128,
                                   bass.ds(col0, 128)])
                for nn in range(2):
                    ps = pso.tile([128, 512], F32, tag="pso")
                    for hb in range(HC):
                        nc.tensor.matmul(
                            ps[:], gath[:, hb * 128:(hb + 1) * 128],
                            wo_sb[hb][:, nn * 512:(nn + 1) * 512],
                            start=(hb == 0), stop=(hb == HC - 1))
                    t_out = outp.tile([128, 512], F32, tag="tout")
                    nc.vector.tensor_copy(t_out[:], ps[:])
                    nc.sync.dma_start(
                        out[qc * 128:(qc + 1) * 128,
                            nn * 512:(nn + 1) * 512], t_out[:])

    nc.compile()
    return nc


_NC_CACHE = {}


def _get_program(causal: bool = True):
    if causal not in _NC_CACHE:
        _NC_CACHE[causal] = build_program(causal)
    return _NC_CACHE[causal]


def _detect_causal(attention_mask):
    """True if everything at or above the 512-block diagonal's upper edge is
    masked off hard enough that exp(mask) == 0 for our purposes."""
    m = np.asarray(attention_mask)  # [B, 1, S(q), S(k)]
    for qc in range(QC):
        k0 = (qc + 1) * 512
        if k0 >= S:
            continue
        blk = m[:, 0, qc * 512:(qc + 1) * 512, k0:]
        if not np.all(blk < -30.0):
            return False
    return True


def make_in_maps(hidden_states, attention_mask, position_ids, cos, sin,
                 Wq, Wk, Wv, Wo, causal):
    import ml_dtypes
    bf16 = ml_dtypes.bfloat16
    hidden_states = np.asarray(hidden_states, np.float32)
    attention_mask = np.asarray(attention_mask, np.float32)
    position_ids = np.asarray(position_ids)
    cos = np.asarray(cos, np.float32)
    sin = np.asarray(sin, np.float32)
    wq_f = np.asarray(Wq, np.float32) * SCALE
    wk_f = np.asarray(Wk, np.float32)
    wv_f = np.asarray(Wv, np.float32)
    wo_ = np.ascontiguousarray(np.asarray(Wo, np.float32)).astype(np.float16)

    in_maps = []
    for b in range(B):
        hsT_b = np.ascontiguousarray(hidden_states[b].T).astype(bf16)
        cos_b = cos[position_ids[b]]  # [S, HD]
        sin_b = sin[position_ids[b]]
        cosT = np.tile(cos_b.T, (2, 1)).astype(bf16)  # [128, S]
        # signed sin: the device swaps k's 32-row halves (rotate-half), so the
        # table stays index-aligned and only carries rotate-half's sign
        sin64 = sin_b.T  # [64, S]
        sh = np.empty_like(sin64)
        sh[0:32] = -sin64[0:32]
        sh[32:64] = sin64[32:64]
        sinT = np.tile(sh, (2, 1)).astype(bf16)  # [128, S]
        mask_b = attention_mask[b, 0]  # [S(q), S(k)]
        if causal:
            em = np.empty((S, 1024), np.float16)
            for qc in range(QC):
                blk = mask_b[qc * 512:(qc + 1) * 512,
                             qc * 512:(qc + 1) * 512].T  # [k, q]
                e = np.exp(blk).astype(np.float16)
                em[qc * 512:(qc + 1) * 512, 0:512] = e
                em[qc * 512:(qc + 1) * 512, 512:1024] = e
        else:
            em = np.exp(mask_b.T).astype(np.float16)  # [k, q]
        for g in range(CPB):
            c0 = g * HPC * HD
            in_maps.append({
                "hsT": hsT_b, "cosk": cosT, "sink": sinT, "emask": em,
                "wq": np.ascontiguousarray(wq_f[:, c0:c0 + HPC * HD]).astype(bf16),
                "wk": np.ascontiguousarray(wk_f[:, c0:c0 + HPC * HD]).astype(bf16),
                "wv": np.ascontiguousarray(wv_f[:, c0:c0 + HPC * HD]).astype(bf16),
                "wo": wo_,
            })
    return in_maps


def run(inputs: dict, trace: bool = False):
    causal = _detect_causal(inputs["attention_mask"])
    nc = _get_program(causal)
    in_maps = make_in_maps(**inputs, causal=causal)
    res = run_bass_kernel_spmd(nc, in_maps, list(range(N_CORES)), trace=trace)
    out = np.empty((B, S, HID), np.float32)
    for c in range(N_CORES):
        b, r = c // CPB, c % CPB
        for qc in range(QC):
            q0 = qc * 512 + r * 128
            out[b, q0:q0 + 128, :] = res.results[c]["out"][qc * 128:(qc + 1) * 128]
    return out, res


def kernel(**inputs) -> np.ndarray:
    out, _ = run(inputs, trace=False)
    return out
